# revision 18
# baseline (speedup 1.0000x reference)
"""Trainium2 Bass kernel for nn_MultiHeadAttention_76295799046818.

MHA: B=2, S=2048, D=1024, H=16 heads (d_k=64), causal, fp32 reference.
Sharded over 8 NeuronCores: data-parallel over batch (2) x tensor-parallel
over heads (4 heads/core).  Wq/Wk/Wv column-parallel; Wo row-parallel with
the 4 partial outputs per batch summed on the host (cheaper than an
on-device all-reduce in this runtime).

Single fused pipeline per core (bf16 matmuls, fp32 PSUM):
  - Projections are chunked by 512 rows of S and interleaved with attention:
    chunk sc feeds attention q-chunk qc=sc, so QK/exp start ~10us in while
    later chunks still stream from HBM.
  - Per (head-pair hp, q-chunk qc): software-pipelined k-tile loop
    QK(ki+1) || exp(ki) || AV(ki-1).  The pair's two heads sit at SBUF
    partitions 0-63/64-127 so their K=64 QK matmuls run on disjoint PE row
    groups concurrently.  exp is sliced to live columns; the causal
    staircase inside diagonal tiles is zeroed by gpsimd affine_select on
    the bf16 exp tile (no PE mask matmuls, no mask DMA).
  - AV accumulates [V|ones] / [ones|0|V] weights so softmax denominators
    land at PSUM rows 64 (even head) / 0 (odd head); C^T raw is copied out
    bf16 immediately (frees PSUM), denominators go through DVE
    reciprocal_approx_fast, a one-hot PE matmul broadcasts 1/den across
    partitions, and a DVE multiply normalizes - all one pipeline step
    behind the sweep.
  - Wo partials per q-chunk trail one more step behind, overlapped with the
    next chunk's attention; output bias on DVE, bf16 DMA out.
"""

import numpy as np
import ml_dtypes

import concourse.bass as bass
import concourse.mybir as mybir
import concourse.tile as tile
from concourse import bacc
from concourse.bass_utils import run_bass_kernel_spmd

BF16 = ml_dtypes.bfloat16

B, S, D, H, DK = 2, 2048, 1024, 16, 64
N_CORES = 8
TP = 4  # head-parallel degree (per batch)
HPC = H // TP  # heads per core = 4
O = HPC * DK  # output channels per core = 256
QT_BLK = 512
N_QC = S // QT_BLK  # 4
KC = D // 128  # 8 contraction chunks for projections

_CACHE = {}


def _build():
    nc = bacc.Bacc("TRN2", target_bir_lowering=False, debug=False,
                   num_devices=N_CORES)
    dt = mybir.dt
    f32, bf16, f32r = dt.float32, dt.bfloat16, dt.float32r

    def din(name, shape, dtype=bf16):
        return nc.dram_tensor(name, shape, dtype, kind="ExternalInput").ap()

    xqt_d = din("xqt", [N_QC, 128, KC, QT_BLK])
    xkt_d = din("xkt", [N_QC, 128, KC, QT_BLK])
    xvt_d = din("xvt", [N_QC, 128, KC, QT_BLK])
    wqt_d = din("wqt", [128, KC, O])
    wkt_d = din("wkt", [128, KC, O])
    wvt_d = din("wvt", [128, KC, O])
    wot_d = din("wot", [128, 2, D])
    bq_d = din("bqc", [128, 2], f32)
    bk_d = din("bkc", [128, 2], f32)
    bvb_d = din("bvb", [128, O], f32)
    bo_d = din("boc", [128, 8], f32)
    bsel_d = din("bsel", [65, 128], f32r)
    out_d = nc.dram_tensor("out", [8, 128, S], bf16,
                           kind="ExternalOutput").ap()

    EXPF = mybir.ActivationFunctionType.Exp
    IDF = mybir.ActivationFunctionType.Identity

    with tile.TileContext(nc) as tc:
        with (
            tc.tile_pool(name="const", bufs=1) as cpool,
            tc.tile_pool(name="xin", bufs=2) as xpool,
            tc.tile_pool(name="expp", bufs=5) as epool,
            tc.tile_pool(name="crp", bufs=3) as crpool,
            tc.tile_pool(name="ctp", bufs=2) as ctpool,
            tc.tile_pool(name="outp", bufs=4) as opool,
        ):
            # hot-path weights + first x chunks, in priority order: the K
            # projection of chunk 0 gates everything, so its DMAs go first
            wk_sb = cpool.tile([128, KC, O], bf16, name="wk_sb")
            nc.sync.dma_start(wk_sb[:, 0:4, :], wkt_d[:, 0:4, :])
            nc.sync.dma_start(wk_sb[:, 4:8, :], wkt_d[:, 4:8, :])
            xk0 = xpool.tile([128, KC, QT_BLK], bf16, name="xk", tag="xk")
            nc.sync.dma_start(xk0[:, 0:2, :], xkt_d[0][:, 0:2, :])
            nc.sync.dma_start(xk0[:, 2:4, :], xkt_d[0][:, 2:4, :])
            nc.sync.dma_start(xk0[:, 4:6, :], xkt_d[0][:, 4:6, :])
            nc.sync.dma_start(xk0[:, 6:8, :], xkt_d[0][:, 6:8, :])
            wq_sb = cpool.tile([128, KC, O], bf16, name="wq_sb")
            nc.sync.dma_start(wq_sb[:, 0:4, :], wqt_d[:, 0:4, :])
            nc.sync.dma_start(wq_sb[:, 4:8, :], wqt_d[:, 4:8, :])
            xq0 = xpool.tile([128, KC, QT_BLK], bf16, name="xq", tag="xq")
            nc.sync.dma_start(xq0[:, 0:4, :], xqt_d[0][:, 0:4, :])
            nc.sync.dma_start(xq0[:, 4:8, :], xqt_d[0][:, 4:8, :])
            wv_sb = cpool.tile([128, KC, O], bf16, name="wv_sb")
            nc.sync.dma_start(wv_sb[:, 0:4, :], wvt_d[:, 0:4, :])
            nc.sync.dma_start(wv_sb[:, 4:8, :], wvt_d[:, 4:8, :])
            xv0 = xpool.tile([128, KC, QT_BLK], bf16, name="xv", tag="xv")
            nc.sync.dma_start(xv0[:, 0:4, :], xvt_d[0][:, 0:4, :])
            nc.sync.dma_start(xv0[:, 4:8, :], xvt_d[0][:, 4:8, :])
            bq_sb = cpool.tile([128, 2], f32, name="bq_sb")
            nc.sync.dma_start(bq_sb[:], bq_d[:])
            bk_sb = cpool.tile([128, 2], f32, name="bk_sb")
            nc.sync.dma_start(bk_sb[:], bk_d[:])
            bvb_sb = cpool.tile([128, O], f32, name="bvb_sb")
            nc.sync.dma_start(bvb_sb[:], bvb_d[:])
            bsel_sb = cpool.tile([65, 128], f32r, name="bsel_sb")
            nc.sync.dma_start(bsel_sb[:], bsel_d[:])
            wo_sb = cpool.tile([128, 2, D], bf16, name="wo_sb")
            nc.sync.dma_start(wo_sb[:, 0, :], wot_d[:, 0, :])
            nc.sync.dma_start(wo_sb[:, 1, :], wot_d[:, 1, :])
            bo_sb = cpool.tile([128, 8], f32, name="bo_sb")
            nc.sync.dma_start(bo_sb[:], bo_d[:])

            qt_sb = cpool.tile([128, 2, S], bf16, name="qt_sb")
            kt_sb = cpool.tile([128, 2, S], bf16, name="kt_sb")
            # AV weights: per k-tile/pair, even head [V|ones] (den @ row 64),
            # odd head [ones|0|V] (den @ row 0, C^T @ rows 64-127)
            vaug_e = cpool.tile([128, 16, 2, 66], bf16, name="vaug_e")
            nc.vector.memset(vaug_e[:], 1.0)
            vaug_o = cpool.tile([128, 16, 2, 128], bf16, name="vaug_o")
            nc.vector.memset(vaug_o[:], 0.0)
            nc.vector.memset(vaug_o[:, :, :, 0:1], 1.0)
            # raw-denominator staging rows 0 (odd head) / 64 (even head);
            # rows 1-63 stay 1.0 (multiplied by bsel zeros in the bcast).
            # memset can't emit f32r, so memset f32 and cast-copy once.
            onesf = cpool.tile([65, QT_BLK], f32, name="onesf")
            nc.vector.memset(onesf[:], 1.0)
            dsb = cpool.tile([65, 2, QT_BLK], f32r, name="dsb")
            nc.vector.tensor_copy(dsb[:, 0, :], onesf[:])
            nc.vector.tensor_copy(dsb[:, 1, :], onesf[:])

            xq_t, xk_t, xv_t = {}, {}, {}

            def dma_chunk(sc):
                xk = xpool.tile([128, KC, QT_BLK], bf16, name="xk", tag="xk")
                nc.sync.dma_start(xk[:, 0:4, :], xkt_d[sc][:, 0:4, :])
                nc.sync.dma_start(xk[:, 4:8, :], xkt_d[sc][:, 4:8, :])
                xq = xpool.tile([128, KC, QT_BLK], bf16, name="xq", tag="xq")
                nc.sync.dma_start(xq[:, 0:4, :], xqt_d[sc][:, 0:4, :])
                nc.sync.dma_start(xq[:, 4:8, :], xqt_d[sc][:, 4:8, :])
                xv = xpool.tile([128, KC, QT_BLK], bf16, name="xv", tag="xv")
                nc.sync.dma_start(xv[:, 0:4, :], xvt_d[sc][:, 0:4, :])
                nc.sync.dma_start(xv[:, 4:8, :], xvt_d[sc][:, 4:8, :])
                xk_t[sc], xq_t[sc], xv_t[sc] = xk, xq, xv

            with tc.tile_pool(name="ps", bufs=2, space="PSUM") as ps:

                def proj(sc):
                    ssl = bass.ds(sc * QT_BLK, QT_BLK)
                    xk, xq, xv = xk_t[sc], xq_t[sc], xv_t[sc]
                    for ot in range(2):
                        pk = ps.tile([128, QT_BLK], f32, name="pk",
                                     tag="big", bufs=2)
                        for kc in range(KC):
                            nc.tensor.matmul(pk[:],
                                             wk_sb[:, kc, bass.ds(ot * 128, 128)],
                                             xk[:, kc, :], start=(kc == 0),
                                             stop=(kc == KC - 1))
                        nc.scalar.activation(kt_sb[:, ot, ssl], pk[:], IDF,
                                             bias=bk_sb[:, ot:ot + 1])
                    for ot in range(2):
                        pq = ps.tile([128, QT_BLK], f32, name="pq",
                                     tag="big", bufs=2)
                        for kc in range(KC):
                            nc.tensor.matmul(pq[:],
                                             wq_sb[:, kc, bass.ds(ot * 128, 128)],
                                             xq[:, kc, :], start=(kc == 0),
                                             stop=(kc == KC - 1))
                        nc.scalar.activation(qt_sb[:, ot, ssl], pq[:], IDF,
                                             bias=bq_sb[:, ot:ot + 1])
                    for mtp in range(2):
                        pv = ps.tile([128, QT_BLK], f32, name="pv",
                                     tag="big", bufs=2)
                        for mt2 in range(2):
                            for kc in range(KC):
                                nc.tensor.matmul(
                                    pv[:, bass.ds(mt2 * O, O)],
                                    xv[:, kc, bass.ds((2 * mtp + mt2) * 128, 128)],
                                    wv_sb[:, kc, :], start=(kc == 0),
                                    stop=(kc == KC - 1))
                        # pv view: [128, (mt2 2, hp 2, two 2, d 64)]
                        pvr = pv[:].rearrange("p (mt hp two d) -> p mt hp two d",
                                              mt=2, hp=2, two=2)
                        bvr = bvb_sb[:].rearrange("p (hp two d) -> p hp two d",
                                                  hp=2, two=2)
                        for mt2 in range(2):
                            kt = sc * 4 + 2 * mtp + mt2
                            nc.vector.tensor_tensor(
                                vaug_e[:, kt, :, 0:64],
                                pvr[:, mt2, :, 0, :], bvr[:, :, 0, :],
                                mybir.AluOpType.add)
                            nc.vector.tensor_tensor(
                                vaug_o[:, kt, :, 64:128],
                                pvr[:, mt2, :, 1, :], bvr[:, :, 1, :],
                                mybir.AluOpType.add)

                xk_t[0], xq_t[0], xv_t[0] = xk0, xq0, xv0
                dma_chunk(1)
                proj(0)

                ct_t = {}
                prev = None

                def bcast_norm(state):
                    # broadcast raw dens across partitions (PE), reciprocal
                    # in SBUF at partition base 0 (recip_approx_fast only
                    # works there), then normalize ctraw -> ct
                    qc, hp, ctraw, pp = state
                    pdup = ps.tile([128, QT_BLK], f32, name="pdup",
                                   tag="big", bufs=2)
                    nc.tensor.matmul(pdup[:], bsel_sb[:], dsb[0:65, pp, :],
                                     start=True, stop=True)
                    pbcs = crpool.tile([128, QT_BLK], f32, name="pbcs",
                                       tag="pbcs")
                    nc.vector.tensor_copy(pbcs[:], pdup[:])
                    pbcr = crpool.tile([128, QT_BLK], f32, name="pbcr",
                                       tag="pbcr")
                    nc.vector.reciprocal_approx_fast(pbcr[:], pbcs[:])
                    if hp == 0:
                        ct = ctpool.tile([128, 2, QT_BLK], bf16, name="ct",
                                         tag="ct")
                        ct_t[qc] = ct
                    ct = ct_t[qc]
                    nc.vector.tensor_tensor(ct[:, hp, :], ctraw[:, :],
                                            pbcr[:, :], mybir.AluOpType.mult)

                def wo(qc):
                    ct = ct_t[qc]
                    qsl = bass.ds(qc * QT_BLK, QT_BLK)
                    for jt in range(8):
                        pwo = ps.tile([128, QT_BLK], f32, name="pwo",
                                      tag="big", bufs=2)
                        for kc in range(2):
                            nc.tensor.matmul(
                                pwo[:], wo_sb[:, kc, bass.ds(jt * 128, 128)],
                                ct[:, kc, :], start=(kc == 0), stop=(kc == 1))
                        osb = opool.tile([128, QT_BLK], bf16, name="osb",
                                         tag="osb")
                        nc.scalar.activation(osb[:], pwo[:], IDF,
                                             bias=bo_sb[:, jt:jt + 1])
                        nc.sync.dma_start(out_d[jt][:, qsl], osb[:])

                for qc in range(N_QC):
                    for hp in range(2):
                        n_ki = 4 * qc + 4
                        pav_e = ps.tile([65, QT_BLK], f32, name="pav_e",
                                        tag="pave", bufs=1)
                        pav_o = ps.tile([128, QT_BLK], f32, name="pav_o",
                                        tag="pavo", bufs=1)

                        def av(idx, ki, et, lo):
                            alo = 0 if idx == 0 else lo
                            nc.tensor.matmul(
                                pav_e[:, alo:QT_BLK],
                                vaug_e[:, ki, hp, 0:65],
                                et[:, 0, alo:QT_BLK],
                                start=(idx == 0), stop=(idx == n_ki - 1),
                                skip_group_check=True)
                            nc.tensor.matmul(
                                pav_o[:, alo:QT_BLK],
                                vaug_o[:, ki, hp, :],
                                et[:, 1, alo:QT_BLK],
                                start=(idx == 0), stop=(idx == n_ki - 1),
                                skip_group_check=True)

                        # software pipeline 2 deep: AV(ki-2) runs while
                        # exp(ki-1) / affine(ki-1) and QK(ki) are in flight,
                        # hiding the slow gpsimd semaphore hop
                        pend = []
                        for ki in range(n_ki):
                            lo = max(0, 128 * ki - QT_BLK * qc)
                            st = ps.tile([128, 2, QT_BLK], f32, name="st",
                                         tag="st", bufs=2)
                            for side in range(2):
                                po = bass.ds(side * 64, 64)
                                nc.tensor.matmul(
                                    st[:, side, lo:QT_BLK],
                                    kt_sb[po, hp, bass.ds(ki * 128, 128)],
                                    qt_sb[po, hp,
                                          bass.ds(qc * QT_BLK + lo,
                                                  QT_BLK - lo)],
                                    start=True, stop=True)
                            et = epool.tile([128, 2, QT_BLK], bf16,
                                            name="et", tag="et")
                            nc.scalar.activation(et[:, :, lo:QT_BLK],
                                                 st[:, :, lo:QT_BLK], EXPF,
                                                 scale=0.125)
                            if ki >= 4 * qc:
                                # zero the causal staircase (cols lo..lo+127)
                                nc.gpsimd.affine_select(
                                    out=et[:, :, bass.ds(lo, 128)],
                                    in_=et[:, :, bass.ds(lo, 128)],
                                    compare_op=mybir.AluOpType.is_ge,
                                    fill=0.0, base=0,
                                    pattern=[[0, 2], [1, 128]],
                                    channel_multiplier=-1)
                            pend.append((ki, ki, et, lo))
                            if len(pend) > 2:
                                av(*pend.pop(0))
                        for p_ in pend:
                            av(*p_)

                        # denominators + raw C^T out (frees pav quickly)
                        ctraw = crpool.tile([128, QT_BLK], bf16,
                                            name="ctraw", tag="ctraw")
                        nc.vector.tensor_copy(ctraw[0:64, :], pav_e[0:64, :])
                        nc.vector.tensor_copy(ctraw[64:128, :],
                                              pav_o[64:128, :])
                        pp = hp
                        nc.vector.tensor_copy(dsb[64:65, pp, :],
                                              pav_e[64:65, :])
                        nc.vector.tensor_copy(dsb[0:1, pp, :],
                                              pav_o[0:1, :])
                        cur = (qc, hp, ctraw, pp)
                        if prev is not None:
                            bcast_norm(prev)
                        if hp == 1:
                            # next-chunk projections + previous chunk's Wo
                            # go after both sweeps so a late input DMA can't
                            # head-of-line-block the ready hp=1 sweep
                            if qc < 3:
                                if qc + 2 < N_QC:
                                    dma_chunk(qc + 2)
                                proj(qc + 1)
                            if qc > 0:
                                wo(qc - 1)
                        prev = cur

                bcast_norm(prev)
                wo(3)

    nc.compile()
    return nc


def kernel(query, key, value, mask, Wq, bq, Wk, bk, Wv, bv, Wo, bo):
    query = np.asarray(query, np.float32)
    key_ = np.asarray(key, np.float32)
    value = np.asarray(value, np.float32)
    Wq, Wk, Wv, Wo = (np.asarray(w, np.float32) for w in (Wq, Wk, Wv, Wo))
    bq, bk, bv, bo = (np.asarray(b_, np.float32) for b_ in (bq, bk, bv, bo))

    mask = np.asarray(mask)
    assert np.array_equal(mask != 0, np.tril(np.ones((S, S), bool))), \
        "kernel is specialized to the causal mask"
    if "nc" not in _CACHE:
        _CACHE["nc"] = _build()
    nc = _CACHE["nc"]

    def xt(x):  # [S, D] -> [N_QC, 128, KC, QT_BLK] bf16, partition-major
        a = x.T.reshape(KC, 128, S).transpose(1, 0, 2)  # [128, KC, S]
        a = a.reshape(128, KC, N_QC, QT_BLK).transpose(2, 0, 1, 3)
        return np.ascontiguousarray(a).astype(BF16)

    def wslice(W, c):  # [D, D] -> [128, KC, O] bf16 of W[o_slice].T
        hg = c % TP
        a = W[hg * O:(hg + 1) * O].T.reshape(KC, 128, O).transpose(1, 0, 2)
        return np.ascontiguousarray(a).astype(BF16)

    # bcast selector: rows 0-63 of pbc take 1/den_even (rdsb row 64),
    # rows 64-127 take 1/den_odd (rdsb row 0)
    bsel = np.zeros((65, 128), np.float32)
    bsel[64, 0:64] = 1.0
    bsel[0, 64:128] = 1.0

    in_maps = []
    for c in range(N_CORES):
        b_, hg = c // TP, c % TP
        osl = slice(hg * O, (hg + 1) * O)
        bo_part = bo if hg == 0 else np.zeros_like(bo)
        wot = Wo[:, osl].T.reshape(2, 128, D).transpose(1, 0, 2)
        in_maps.append({
            "xqt": xt(query[b_]),
            "xkt": xt(key_[b_]),
            "xvt": xt(value[b_]),
            "wqt": wslice(Wq, c),
            "wkt": wslice(Wk, c),
            "wvt": wslice(Wv, c),
            "wot": np.ascontiguousarray(wot).astype(BF16),
            "bqc": np.ascontiguousarray(bq[osl].reshape(2, 128).T),
            "bkc": np.ascontiguousarray(bk[osl].reshape(2, 128).T),
            "bvb": np.ascontiguousarray(np.broadcast_to(bv[osl], (128, O))),
            "boc": np.ascontiguousarray(bo_part.reshape(8, 128).T),
            "bsel": bsel,
        })

    res = run_bass_kernel_spmd(nc, in_maps, core_ids=list(range(N_CORES)))

    out = np.zeros((B, S, D), np.float32)
    for c in range(N_CORES):
        part = res.results[c]["out"].reshape(D, S)  # out^T [j, s]
        out[c // TP] += part.T.astype(np.float32)
    return out


# revision 21
# speedup vs baseline: 1.1685x; 1.1685x over previous
"""Trainium2 Bass kernel for nn_MultiHeadAttention_76295799046818.

MHA: B=2, S=2048, D=1024, H=16 heads (d_k=64), causal, fp32 reference.
Sharded over 8 NeuronCores: data-parallel over batch (2) x tensor-parallel
over heads (4 heads/core).  Wq/Wk/Wv column-parallel; Wo row-parallel with
the 4 partial outputs per batch summed on the host (cheaper than an
on-device all-reduce in this runtime).

Single fused pipeline per core (bf16 matmuls, fp32 PSUM):
  - Projections are chunked by 512 rows of S and interleaved with attention:
    chunk sc feeds attention q-chunk qc=sc, so QK/exp start ~10us in while
    later chunks still stream from HBM.
  - Per (head-pair hp, q-chunk qc): software-pipelined k-tile loop
    QK(ki+1) || exp(ki) || AV(ki-1).  The pair's two heads sit at SBUF
    partitions 0-63/64-127 so their K=64 QK matmuls run on disjoint PE row
    groups concurrently.  exp is sliced to live columns; the causal
    staircase inside diagonal tiles is zeroed by gpsimd affine_select on
    the bf16 exp tile (no PE mask matmuls, no mask DMA).
  - AV accumulates [V|ones] / [ones|0|V] weights so softmax denominators
    land at PSUM rows 64 (even head) / 0 (odd head); C^T raw is copied out
    bf16 immediately (frees PSUM), denominators go through DVE
    reciprocal_approx_fast, a one-hot PE matmul broadcasts 1/den across
    partitions, and a DVE multiply normalizes - all one pipeline step
    behind the sweep.
  - Wo partials per q-chunk trail one more step behind, overlapped with the
    next chunk's attention; output bias on DVE, bf16 DMA out.
"""

import numpy as np
import ml_dtypes

import concourse.bass as bass
import concourse.mybir as mybir
import concourse.tile as tile
from concourse import bacc
from concourse.bass_utils import run_bass_kernel_spmd

BF16 = ml_dtypes.bfloat16

B, S, D, H, DK = 2, 2048, 1024, 16, 64
N_CORES = 8
TP = 4  # head-parallel degree (per batch)
HPC = H // TP  # heads per core = 4
O = HPC * DK  # output channels per core = 256
QT_BLK = 512
N_QC = S // QT_BLK  # 4
KC = D // 128  # 8 contraction chunks for projections

_CACHE = {}


def _build():
    nc = bacc.Bacc("TRN2", target_bir_lowering=False, debug=False,
                   num_devices=N_CORES)
    dt = mybir.dt
    f32, bf16, f32r = dt.float32, dt.bfloat16, dt.float32r

    def din(name, shape, dtype=bf16):
        return nc.dram_tensor(name, shape, dtype, kind="ExternalInput").ap()

    xqt_d = din("xqt", [N_QC, 128, KC, QT_BLK])
    xkt_d = din("xkt", [N_QC, 128, KC, QT_BLK])
    xvt_d = din("xvt", [N_QC, 128, KC, QT_BLK])
    wqt_d = din("wqt", [128, KC, O])
    wkt_d = din("wkt", [128, KC, O])
    wvt_d = din("wvt", [128, KC, O])
    wot_d = din("wot", [128, 2, D])
    bq_d = din("bqc", [128, 2], f32)
    bk_d = din("bkc", [128, 2], f32)
    bvb_d = din("bvb", [128, O], f32)
    bo_d = din("boc", [128, 8], f32)
    bsel_d = din("bsel", [65, 128], f32r)
    out_d = nc.dram_tensor("out", [8, 128, S], bf16,
                           kind="ExternalOutput").ap()

    EXPF = mybir.ActivationFunctionType.Exp
    IDF = mybir.ActivationFunctionType.Identity

    with tile.TileContext(nc) as tc:
        with (
            tc.tile_pool(name="const", bufs=1) as cpool,
            tc.tile_pool(name="xin", bufs=2) as xpool,
            tc.tile_pool(name="expp", bufs=5) as epool,
            tc.tile_pool(name="crp", bufs=3) as crpool,
            tc.tile_pool(name="ctp", bufs=2) as ctpool,
            tc.tile_pool(name="outp", bufs=4) as opool,
        ):
            # hot-path weights + first x chunks, in priority order: the K
            # projection of chunk 0 gates everything, so its DMAs go first
            wk_sb = cpool.tile([128, KC, O], bf16, name="wk_sb")
            nc.sync.dma_start(wk_sb[:, 0:4, :], wkt_d[:, 0:4, :])
            nc.sync.dma_start(wk_sb[:, 4:8, :], wkt_d[:, 4:8, :])
            xk0 = xpool.tile([128, KC, QT_BLK], bf16, name="xk", tag="xk")
            nc.sync.dma_start(xk0[:, 0:2, :], xkt_d[0][:, 0:2, :])
            nc.sync.dma_start(xk0[:, 2:4, :], xkt_d[0][:, 2:4, :])
            nc.sync.dma_start(xk0[:, 4:6, :], xkt_d[0][:, 4:6, :])
            nc.sync.dma_start(xk0[:, 6:8, :], xkt_d[0][:, 6:8, :])
            wq_sb = cpool.tile([128, KC, O], bf16, name="wq_sb")
            nc.sync.dma_start(wq_sb[:, 0:4, :], wqt_d[:, 0:4, :])
            nc.sync.dma_start(wq_sb[:, 4:8, :], wqt_d[:, 4:8, :])
            xq0 = xpool.tile([128, KC, QT_BLK], bf16, name="xq", tag="xq")
            nc.sync.dma_start(xq0[:, 0:4, :], xqt_d[0][:, 0:4, :])
            nc.sync.dma_start(xq0[:, 4:8, :], xqt_d[0][:, 4:8, :])
            # chunk-1 K/Q next: proj(1) gates attention qc>=1, and only
            # V is needed before the first AV
            xk1 = xpool.tile([128, KC, QT_BLK], bf16, name="xk", tag="xk")
            nc.sync.dma_start(xk1[:, 0:4, :], xkt_d[1][:, 0:4, :])
            nc.sync.dma_start(xk1[:, 4:8, :], xkt_d[1][:, 4:8, :])
            xq1 = xpool.tile([128, KC, QT_BLK], bf16, name="xq", tag="xq")
            nc.sync.dma_start(xq1[:, 0:4, :], xqt_d[1][:, 0:4, :])
            nc.sync.dma_start(xq1[:, 4:8, :], xqt_d[1][:, 4:8, :])
            wv_sb = cpool.tile([128, KC, O], bf16, name="wv_sb")
            nc.sync.dma_start(wv_sb[:, 0:4, :], wvt_d[:, 0:4, :])
            nc.sync.dma_start(wv_sb[:, 4:8, :], wvt_d[:, 4:8, :])
            xv0 = xpool.tile([128, KC, QT_BLK], bf16, name="xv", tag="xv")
            nc.sync.dma_start(xv0[:, 0:4, :], xvt_d[0][:, 0:4, :])
            nc.sync.dma_start(xv0[:, 4:8, :], xvt_d[0][:, 4:8, :])
            xv1 = xpool.tile([128, KC, QT_BLK], bf16, name="xv", tag="xv")
            nc.sync.dma_start(xv1[:, 0:4, :], xvt_d[1][:, 0:4, :])
            nc.sync.dma_start(xv1[:, 4:8, :], xvt_d[1][:, 4:8, :])
            bq_sb = cpool.tile([128, 2], f32, name="bq_sb")
            nc.sync.dma_start(bq_sb[:], bq_d[:])
            bk_sb = cpool.tile([128, 2], f32, name="bk_sb")
            nc.sync.dma_start(bk_sb[:], bk_d[:])
            bvb_sb = cpool.tile([128, O], f32, name="bvb_sb")
            nc.sync.dma_start(bvb_sb[:], bvb_d[:])
            bsel_sb = cpool.tile([65, 128], f32r, name="bsel_sb")
            nc.sync.dma_start(bsel_sb[:], bsel_d[:])
            wo_sb = cpool.tile([128, 2, D], bf16, name="wo_sb")
            nc.sync.dma_start(wo_sb[:, 0, :], wot_d[:, 0, :])
            nc.sync.dma_start(wo_sb[:, 1, :], wot_d[:, 1, :])
            bo_sb = cpool.tile([128, 8], f32, name="bo_sb")
            nc.sync.dma_start(bo_sb[:], bo_d[:])

            qt_sb = cpool.tile([128, 2, S], bf16, name="qt_sb")
            kt_sb = cpool.tile([128, 2, S], bf16, name="kt_sb")
            # AV weights: per k-tile/pair, even head [V|ones] (den @ row 64),
            # odd head [ones|0|V] (den @ row 0, C^T @ rows 64-127)
            vaug_e = cpool.tile([128, 16, 2, 66], bf16, name="vaug_e")
            nc.vector.memset(vaug_e[:], 1.0)
            vaug_o = cpool.tile([128, 16, 2, 128], bf16, name="vaug_o")
            nc.vector.memset(vaug_o[:], 0.0)
            nc.vector.memset(vaug_o[:, :, :, 0:1], 1.0)
            # raw-denominator staging rows 0 (odd head) / 64 (even head);
            # rows 1-63 stay 1.0 (multiplied by bsel zeros in the bcast).
            # memset can't emit f32r, so memset f32 and cast-copy once.
            onesf = cpool.tile([65, QT_BLK], f32, name="onesf")
            nc.vector.memset(onesf[:], 1.0)
            dsb = cpool.tile([65, 2, QT_BLK], f32r, name="dsb")
            nc.vector.tensor_copy(dsb[:, 0, :], onesf[:])
            nc.vector.tensor_copy(dsb[:, 1, :], onesf[:])

            xq_t, xk_t, xv_t = {}, {}, {}

            def dma_chunk(sc):
                xk = xpool.tile([128, KC, QT_BLK], bf16, name="xk", tag="xk")
                nc.sync.dma_start(xk[:, 0:4, :], xkt_d[sc][:, 0:4, :])
                nc.sync.dma_start(xk[:, 4:8, :], xkt_d[sc][:, 4:8, :])
                xq = xpool.tile([128, KC, QT_BLK], bf16, name="xq", tag="xq")
                nc.sync.dma_start(xq[:, 0:4, :], xqt_d[sc][:, 0:4, :])
                nc.sync.dma_start(xq[:, 4:8, :], xqt_d[sc][:, 4:8, :])
                xv = xpool.tile([128, KC, QT_BLK], bf16, name="xv", tag="xv")
                nc.sync.dma_start(xv[:, 0:4, :], xvt_d[sc][:, 0:4, :])
                nc.sync.dma_start(xv[:, 4:8, :], xvt_d[sc][:, 4:8, :])
                xk_t[sc], xq_t[sc], xv_t[sc] = xk, xq, xv

            with tc.tile_pool(name="ps", bufs=2, space="PSUM") as ps:

                def proj(sc):
                    ssl = bass.ds(sc * QT_BLK, QT_BLK)
                    xk, xq, xv = xk_t[sc], xq_t[sc], xv_t[sc]
                    for ot in range(2):
                        pk = ps.tile([128, QT_BLK], f32, name="pk",
                                     tag="big", bufs=2)
                        for kc in range(KC):
                            nc.tensor.matmul(pk[:],
                                             wk_sb[:, kc, bass.ds(ot * 128, 128)],
                                             xk[:, kc, :], start=(kc == 0),
                                             stop=(kc == KC - 1))
                        nc.scalar.activation(kt_sb[:, ot, ssl], pk[:], IDF,
                                             bias=bk_sb[:, ot:ot + 1])
                    for ot in range(2):
                        pq = ps.tile([128, QT_BLK], f32, name="pq",
                                     tag="big", bufs=2)
                        for kc in range(KC):
                            nc.tensor.matmul(pq[:],
                                             wq_sb[:, kc, bass.ds(ot * 128, 128)],
                                             xq[:, kc, :], start=(kc == 0),
                                             stop=(kc == KC - 1))
                        nc.scalar.activation(qt_sb[:, ot, ssl], pq[:], IDF,
                                             bias=bq_sb[:, ot:ot + 1])
                    for mtp in range(2):
                        pv = ps.tile([128, QT_BLK], f32, name="pv",
                                     tag="big", bufs=2)
                        for mt2 in range(2):
                            for kc in range(KC):
                                nc.tensor.matmul(
                                    pv[:, bass.ds(mt2 * O, O)],
                                    xv[:, kc, bass.ds((2 * mtp + mt2) * 128, 128)],
                                    wv_sb[:, kc, :], start=(kc == 0),
                                    stop=(kc == KC - 1))
                        # pv view: [128, (mt2 2, hp 2, two 2, d 64)]
                        pvr = pv[:].rearrange("p (mt hp two d) -> p mt hp two d",
                                              mt=2, hp=2, two=2)
                        bvr = bvb_sb[:].rearrange("p (hp two d) -> p hp two d",
                                                  hp=2, two=2)
                        for mt2 in range(2):
                            kt = sc * 4 + 2 * mtp + mt2
                            nc.vector.tensor_tensor(
                                vaug_e[:, kt, :, 0:64],
                                pvr[:, mt2, :, 0, :], bvr[:, :, 0, :],
                                mybir.AluOpType.add)
                            nc.vector.tensor_tensor(
                                vaug_o[:, kt, :, 64:128],
                                pvr[:, mt2, :, 1, :], bvr[:, :, 1, :],
                                mybir.AluOpType.add)

                xk_t[0], xq_t[0], xv_t[0] = xk0, xq0, xv0
                xk_t[1], xq_t[1], xv_t[1] = xk1, xq1, xv1
                proj(0)

                ct_t = {}
                prev = None

                def bcast_norm(state):
                    # broadcast raw dens across partitions (PE), reciprocal
                    # in SBUF at partition base 0 (recip_approx_fast only
                    # works there), then normalize ctraw -> ct
                    qc, hp, ctraw, pp = state
                    pdup = ps.tile([128, QT_BLK], f32, name="pdup",
                                   tag="big", bufs=2)
                    nc.tensor.matmul(pdup[:], bsel_sb[:], dsb[0:65, pp, :],
                                     start=True, stop=True)
                    pbcs = crpool.tile([128, QT_BLK], f32, name="pbcs",
                                       tag="pbcs")
                    nc.vector.tensor_copy(pbcs[:], pdup[:])
                    pbcr = crpool.tile([128, QT_BLK], f32, name="pbcr",
                                       tag="pbcr")
                    nc.vector.reciprocal_approx_fast(pbcr[:], pbcs[:])
                    if hp == 0:
                        ct = ctpool.tile([128, 2, QT_BLK], bf16, name="ct",
                                         tag="ct")
                        ct_t[qc] = ct
                    ct = ct_t[qc]
                    nc.vector.tensor_tensor(ct[:, hp, :], ctraw[:, :],
                                            pbcr[:, :], mybir.AluOpType.mult)

                def wo(qc):
                    ct = ct_t[qc]
                    qsl = bass.ds(qc * QT_BLK, QT_BLK)
                    for jt in range(8):
                        pwo = ps.tile([128, QT_BLK], f32, name="pwo",
                                      tag="big", bufs=2)
                        for kc in range(2):
                            nc.tensor.matmul(
                                pwo[:], wo_sb[:, kc, bass.ds(jt * 128, 128)],
                                ct[:, kc, :], start=(kc == 0), stop=(kc == 1))
                        osb = opool.tile([128, QT_BLK], bf16, name="osb",
                                         tag="osb")
                        nc.scalar.activation(osb[:], pwo[:], IDF,
                                             bias=bo_sb[:, jt:jt + 1])
                        nc.sync.dma_start(out_d[jt][:, qsl], osb[:])

                for qc in range(N_QC):
                    for hp in range(2):
                        n_ki = 4 * qc + 4
                        pav_e = ps.tile([65, QT_BLK], f32, name="pav_e",
                                        tag="pave", bufs=1)
                        pav_o = ps.tile([128, QT_BLK], f32, name="pav_o",
                                        tag="pavo", bufs=1)

                        def av(idx, ki, et, lo):
                            alo = 0 if idx == 0 else lo
                            nc.tensor.matmul(
                                pav_e[:, alo:QT_BLK],
                                vaug_e[:, ki, hp, 0:65],
                                et[:, 0, alo:QT_BLK],
                                start=(idx == 0), stop=(idx == n_ki - 1),
                                skip_group_check=True)
                            nc.tensor.matmul(
                                pav_o[:, alo:QT_BLK],
                                vaug_o[:, ki, hp, :],
                                et[:, 1, alo:QT_BLK],
                                start=(idx == 0), stop=(idx == n_ki - 1),
                                skip_group_check=True)

                        # software pipeline 2 deep: AV(ki-2) runs while
                        # exp(ki-1) / affine(ki-1) and QK(ki) are in flight,
                        # hiding the slow gpsimd semaphore hop
                        pend = []
                        for ki in range(n_ki):
                            lo = max(0, 128 * ki - QT_BLK * qc)
                            st = ps.tile([128, 2, QT_BLK], f32, name="st",
                                         tag="st", bufs=2)
                            for side in range(2):
                                po = bass.ds(side * 64, 64)
                                nc.tensor.matmul(
                                    st[:, side, lo:QT_BLK],
                                    kt_sb[po, hp, bass.ds(ki * 128, 128)],
                                    qt_sb[po, hp,
                                          bass.ds(qc * QT_BLK + lo,
                                                  QT_BLK - lo)],
                                    start=True, stop=True)
                            et = epool.tile([128, 2, QT_BLK], bf16,
                                            name="et", tag="et")
                            nc.scalar.activation(et[:, :, lo:QT_BLK],
                                                 st[:, :, lo:QT_BLK], EXPF,
                                                 scale=0.125)
                            if ki >= 4 * qc:
                                # zero the causal staircase (cols lo..lo+127)
                                nc.gpsimd.affine_select(
                                    out=et[:, :, bass.ds(lo, 128)],
                                    in_=et[:, :, bass.ds(lo, 128)],
                                    compare_op=mybir.AluOpType.is_ge,
                                    fill=0.0, base=0,
                                    pattern=[[0, 2], [1, 128]],
                                    channel_multiplier=-1)
                            pend.append((ki, ki, et, lo))
                            if len(pend) > 2:
                                av(*pend.pop(0))
                        for p_ in pend:
                            av(*p_)

                        # denominators + raw C^T out (frees pav quickly)
                        ctraw = crpool.tile([128, QT_BLK], bf16,
                                            name="ctraw", tag="ctraw")
                        nc.vector.tensor_copy(ctraw[0:64, :], pav_e[0:64, :])
                        nc.vector.tensor_copy(ctraw[64:128, :],
                                              pav_o[64:128, :])
                        pp = hp
                        nc.vector.tensor_copy(dsb[64:65, pp, :],
                                              pav_e[64:65, :])
                        nc.vector.tensor_copy(dsb[0:1, pp, :],
                                              pav_o[0:1, :])
                        cur = (qc, hp, ctraw, pp)
                        if prev is not None:
                            bcast_norm(prev)
                        if hp == 0:
                            if qc < 3:
                                if qc + 2 < N_QC:
                                    dma_chunk(qc + 2)
                                proj(qc + 1)
                        else:
                            if qc > 0:
                                wo(qc - 1)
                        prev = cur

                bcast_norm(prev)
                wo(3)

    nc.compile()
    return nc


def kernel(query, key, value, mask, Wq, bq, Wk, bk, Wv, bv, Wo, bo):
    query = np.asarray(query, np.float32)
    key_ = np.asarray(key, np.float32)
    value = np.asarray(value, np.float32)
    Wq, Wk, Wv, Wo = (np.asarray(w, np.float32) for w in (Wq, Wk, Wv, Wo))
    bq, bk, bv, bo = (np.asarray(b_, np.float32) for b_ in (bq, bk, bv, bo))

    mask = np.asarray(mask)
    assert np.array_equal(mask != 0, np.tril(np.ones((S, S), bool))), \
        "kernel is specialized to the causal mask"
    if "nc" not in _CACHE:
        _CACHE["nc"] = _build()
    nc = _CACHE["nc"]

    def xt(x):  # [S, D] -> [N_QC, 128, KC, QT_BLK] bf16, partition-major
        a = x.T.reshape(KC, 128, S).transpose(1, 0, 2)  # [128, KC, S]
        a = a.reshape(128, KC, N_QC, QT_BLK).transpose(2, 0, 1, 3)
        return np.ascontiguousarray(a).astype(BF16)

    def wslice(W, c):  # [D, D] -> [128, KC, O] bf16 of W[o_slice].T
        hg = c % TP
        a = W[hg * O:(hg + 1) * O].T.reshape(KC, 128, O).transpose(1, 0, 2)
        return np.ascontiguousarray(a).astype(BF16)

    # bcast selector: rows 0-63 of pbc take 1/den_even (rdsb row 64),
    # rows 64-127 take 1/den_odd (rdsb row 0)
    bsel = np.zeros((65, 128), np.float32)
    bsel[64, 0:64] = 1.0
    bsel[0, 64:128] = 1.0

    in_maps = []
    for c in range(N_CORES):
        b_, hg = c // TP, c % TP
        osl = slice(hg * O, (hg + 1) * O)
        bo_part = bo if hg == 0 else np.zeros_like(bo)
        wot = Wo[:, osl].T.reshape(2, 128, D).transpose(1, 0, 2)
        in_maps.append({
            "xqt": xt(query[b_]),
            "xkt": xt(key_[b_]),
            "xvt": xt(value[b_]),
            "wqt": wslice(Wq, c),
            "wkt": wslice(Wk, c),
            "wvt": wslice(Wv, c),
            "wot": np.ascontiguousarray(wot).astype(BF16),
            "bqc": np.ascontiguousarray(bq[osl].reshape(2, 128).T),
            "bkc": np.ascontiguousarray(bk[osl].reshape(2, 128).T),
            "bvb": np.ascontiguousarray(np.broadcast_to(bv[osl], (128, O))),
            "boc": np.ascontiguousarray(bo_part.reshape(8, 128).T),
            "bsel": bsel,
        })

    res = run_bass_kernel_spmd(nc, in_maps, core_ids=list(range(N_CORES)))

    out = np.zeros((B, S, D), np.float32)
    for c in range(N_CORES):
        part = res.results[c]["out"].reshape(D, S)  # out^T [j, s]
        out[c // TP] += part.T.astype(np.float32)
    return out


# revision 23
# speedup vs baseline: 1.2221x; 1.0459x over previous
"""Trainium2 Bass kernel for nn_MultiHeadAttention_76295799046818.

MHA: B=2, S=2048, D=1024, H=16 heads (d_k=64), causal, fp32 reference.
Sharded over 8 NeuronCores: data-parallel over batch (2) x tensor-parallel
over heads (4 heads/core).  Wq/Wk/Wv column-parallel; Wo row-parallel with
the 4 partial outputs per batch summed on the host (cheaper than an
on-device all-reduce in this runtime).

Single fused pipeline per core (bf16 matmuls, fp32 PSUM):
  - Projections are chunked by 512 rows of S and interleaved with attention:
    chunk sc feeds attention q-chunk qc=sc, so QK/exp start ~10us in while
    later chunks still stream from HBM.
  - Per (head-pair hp, q-chunk qc): software-pipelined k-tile loop
    QK(ki+1) || exp(ki) || AV(ki-1).  The pair's two heads sit at SBUF
    partitions 0-63/64-127 so their K=64 QK matmuls run on disjoint PE row
    groups concurrently.  exp is sliced to live columns; the causal
    staircase inside diagonal tiles is zeroed by gpsimd affine_select on
    the bf16 exp tile (no PE mask matmuls, no mask DMA).
  - AV accumulates [V|ones] / [ones|0|V] weights so softmax denominators
    land at PSUM rows 64 (even head) / 0 (odd head); C^T raw is copied out
    bf16 immediately (frees PSUM), denominators go through DVE
    reciprocal_approx_fast, a one-hot PE matmul broadcasts 1/den across
    partitions, and a DVE multiply normalizes - all one pipeline step
    behind the sweep.
  - Wo partials per q-chunk trail one more step behind, overlapped with the
    next chunk's attention; output bias on DVE, bf16 DMA out.
"""

import numpy as np
import ml_dtypes

import concourse.bass as bass
import concourse.mybir as mybir
import concourse.tile as tile
from concourse import bacc
from concourse.bass_utils import run_bass_kernel_spmd

BF16 = ml_dtypes.bfloat16

B, S, D, H, DK = 2, 2048, 1024, 16, 64
N_CORES = 8
TP = 4  # head-parallel degree (per batch)
HPC = H // TP  # heads per core = 4
O = HPC * DK  # output channels per core = 256
QT_BLK = 512
N_QC = S // QT_BLK  # 4
KC = D // 128  # 8 contraction chunks for projections

_CACHE = {}


def _build():
    nc = bacc.Bacc("TRN2", target_bir_lowering=False, debug=False,
                   num_devices=N_CORES)
    dt = mybir.dt
    f32, bf16, f32r = dt.float32, dt.bfloat16, dt.float32r

    def din(name, shape, dtype=bf16):
        return nc.dram_tensor(name, shape, dtype, kind="ExternalInput").ap()

    xqt_d = din("xqt", [N_QC, 128, KC, QT_BLK])
    xkt_d = din("xkt", [N_QC, 128, KC, QT_BLK])
    xvt_d = din("xvt", [N_QC, 128, KC, QT_BLK])
    wqt_d = din("wqt", [128, KC, O])
    wkt_d = din("wkt", [128, KC, O])
    wvt_d = din("wvt", [128, KC, O])
    wot_d = din("wot", [128, 2, D])
    bq_d = din("bqc", [128, 2], f32)
    bk_d = din("bkc", [128, 2], f32)
    bvb_d = din("bvb", [128, O], f32)
    bo_d = din("boc", [128, 8], f32)
    bsel_d = din("bsel", [65, 128], f32r)
    out_d = nc.dram_tensor("out", [8, 128, S], bf16,
                           kind="ExternalOutput").ap()

    EXPF = mybir.ActivationFunctionType.Exp
    IDF = mybir.ActivationFunctionType.Identity

    with tile.TileContext(nc) as tc:
        with (
            tc.tile_pool(name="const", bufs=1) as cpool,
            tc.tile_pool(name="xin", bufs=2) as xpool,
            tc.tile_pool(name="expp", bufs=5) as epool,
            tc.tile_pool(name="crp", bufs=3) as crpool,
            tc.tile_pool(name="ctp", bufs=2) as ctpool,
            tc.tile_pool(name="outp", bufs=4) as opool,
        ):
            # hot-path weights + first x chunks, in priority order: the K
            # projection of chunk 0 gates everything, so its DMAs go first
            wk_sb = cpool.tile([128, KC, O], bf16, name="wk_sb")
            nc.sync.dma_start(wk_sb[:, 0:4, :], wkt_d[:, 0:4, :])
            nc.sync.dma_start(wk_sb[:, 4:8, :], wkt_d[:, 4:8, :])
            xk0 = xpool.tile([128, KC, QT_BLK], bf16, name="xk", tag="xk")
            nc.sync.dma_start(xk0[:, 0:2, :], xkt_d[0][:, 0:2, :])
            nc.sync.dma_start(xk0[:, 2:4, :], xkt_d[0][:, 2:4, :])
            nc.sync.dma_start(xk0[:, 4:6, :], xkt_d[0][:, 4:6, :])
            nc.sync.dma_start(xk0[:, 6:8, :], xkt_d[0][:, 6:8, :])
            wq_sb = cpool.tile([128, KC, O], bf16, name="wq_sb")
            nc.sync.dma_start(wq_sb[:, 0:4, :], wqt_d[:, 0:4, :])
            nc.sync.dma_start(wq_sb[:, 4:8, :], wqt_d[:, 4:8, :])
            xq0 = xpool.tile([128, KC, QT_BLK], bf16, name="xq", tag="xq")
            nc.sync.dma_start(xq0[:, 0:4, :], xqt_d[0][:, 0:4, :])
            nc.sync.dma_start(xq0[:, 4:8, :], xqt_d[0][:, 4:8, :])
            wv_sb = cpool.tile([128, KC, O], bf16, name="wv_sb")
            nc.sync.dma_start(wv_sb[:, 0:4, :], wvt_d[:, 0:4, :])
            nc.sync.dma_start(wv_sb[:, 4:8, :], wvt_d[:, 4:8, :])
            xv0 = xpool.tile([128, KC, QT_BLK], bf16, name="xv", tag="xv")
            nc.sync.dma_start(xv0[:, 0:4, :], xvt_d[0][:, 0:4, :])
            nc.sync.dma_start(xv0[:, 4:8, :], xvt_d[0][:, 4:8, :])
            bq_sb = cpool.tile([128, 2], f32, name="bq_sb")
            nc.sync.dma_start(bq_sb[:], bq_d[:])
            bk_sb = cpool.tile([128, 2], f32, name="bk_sb")
            nc.sync.dma_start(bk_sb[:], bk_d[:])
            bvb_sb = cpool.tile([128, O], f32, name="bvb_sb")
            nc.sync.dma_start(bvb_sb[:], bvb_d[:])
            bsel_sb = cpool.tile([65, 128], f32r, name="bsel_sb")
            nc.sync.dma_start(bsel_sb[:], bsel_d[:])
            wo_sb = cpool.tile([128, 2, D], bf16, name="wo_sb")
            nc.sync.dma_start(wo_sb[:, 0, :], wot_d[:, 0, :])
            nc.sync.dma_start(wo_sb[:, 1, :], wot_d[:, 1, :])
            bo_sb = cpool.tile([128, 8], f32, name="bo_sb")
            nc.sync.dma_start(bo_sb[:], bo_d[:])

            qt_sb = cpool.tile([128, 2, S], bf16, name="qt_sb")
            kt_sb = cpool.tile([128, 2, S], bf16, name="kt_sb")
            # AV weights: per k-tile/pair, even head [V|ones] (den @ row 64),
            # odd head [ones|0|V] (den @ row 0, C^T @ rows 64-127)
            vaug_e = cpool.tile([128, 16, 2, 66], bf16, name="vaug_e")
            nc.vector.memset(vaug_e[:], 1.0)
            vaug_o = cpool.tile([128, 16, 2, 128], bf16, name="vaug_o")
            nc.vector.memset(vaug_o[:], 0.0)
            nc.vector.memset(vaug_o[:, :, :, 0:1], 1.0)
            # raw-denominator staging rows 0 (odd head) / 64 (even head);
            # rows 1-63 stay 1.0 (multiplied by bsel zeros in the bcast).
            # memset can't emit f32r, so memset f32 and cast-copy once.
            onesf = cpool.tile([65, QT_BLK], f32, name="onesf")
            nc.vector.memset(onesf[:], 1.0)
            dsb = cpool.tile([65, 2, QT_BLK], f32r, name="dsb")
            nc.vector.tensor_copy(dsb[:, 0, :], onesf[:])
            nc.vector.tensor_copy(dsb[:, 1, :], onesf[:])

            xq_t, xk_t, xv_t = {}, {}, {}

            def dma_chunk(sc):
                xk = xpool.tile([128, KC, QT_BLK], bf16, name="xk", tag="xk")
                nc.sync.dma_start(xk[:, 0:4, :], xkt_d[sc][:, 0:4, :])
                nc.sync.dma_start(xk[:, 4:8, :], xkt_d[sc][:, 4:8, :])
                xq = xpool.tile([128, KC, QT_BLK], bf16, name="xq", tag="xq")
                nc.sync.dma_start(xq[:, 0:4, :], xqt_d[sc][:, 0:4, :])
                nc.sync.dma_start(xq[:, 4:8, :], xqt_d[sc][:, 4:8, :])
                xv = xpool.tile([128, KC, QT_BLK], bf16, name="xv", tag="xv")
                nc.sync.dma_start(xv[:, 0:4, :], xvt_d[sc][:, 0:4, :])
                nc.sync.dma_start(xv[:, 4:8, :], xvt_d[sc][:, 4:8, :])
                xk_t[sc], xq_t[sc], xv_t[sc] = xk, xq, xv

            with tc.tile_pool(name="ps", bufs=2, space="PSUM") as ps:

                def proj(sc):
                    ssl = bass.ds(sc * QT_BLK, QT_BLK)
                    xk, xq, xv = xk_t[sc], xq_t[sc], xv_t[sc]
                    for ot in range(2):
                        pk = ps.tile([128, QT_BLK], f32, name="pk",
                                     tag="big", bufs=2)
                        for kc in range(KC):
                            nc.tensor.matmul(pk[:],
                                             wk_sb[:, kc, bass.ds(ot * 128, 128)],
                                             xk[:, kc, :], start=(kc == 0),
                                             stop=(kc == KC - 1))
                        nc.scalar.activation(kt_sb[:, ot, ssl], pk[:], IDF,
                                             bias=bk_sb[:, ot:ot + 1])
                    for ot in range(2):
                        pq = ps.tile([128, QT_BLK], f32, name="pq",
                                     tag="big", bufs=2)
                        for kc in range(KC):
                            nc.tensor.matmul(pq[:],
                                             wq_sb[:, kc, bass.ds(ot * 128, 128)],
                                             xq[:, kc, :], start=(kc == 0),
                                             stop=(kc == KC - 1))
                        nc.scalar.activation(qt_sb[:, ot, ssl], pq[:], IDF,
                                             bias=bq_sb[:, ot:ot + 1])
                    for mtp in range(2):
                        pv = ps.tile([128, QT_BLK], f32, name="pv",
                                     tag="big", bufs=2)
                        for mt2 in range(2):
                            for kc in range(KC):
                                nc.tensor.matmul(
                                    pv[:, bass.ds(mt2 * O, O)],
                                    xv[:, kc, bass.ds((2 * mtp + mt2) * 128, 128)],
                                    wv_sb[:, kc, :], start=(kc == 0),
                                    stop=(kc == KC - 1))
                        # pv view: [128, (mt2 2, hp 2, two 2, d 64)]
                        pvr = pv[:].rearrange("p (mt hp two d) -> p mt hp two d",
                                              mt=2, hp=2, two=2)
                        bvr = bvb_sb[:].rearrange("p (hp two d) -> p hp two d",
                                                  hp=2, two=2)
                        for mt2 in range(2):
                            kt = sc * 4 + 2 * mtp + mt2
                            nc.vector.tensor_tensor(
                                vaug_e[:, kt, :, 0:64],
                                pvr[:, mt2, :, 0, :], bvr[:, :, 0, :],
                                mybir.AluOpType.add)
                            nc.vector.tensor_tensor(
                                vaug_o[:, kt, :, 64:128],
                                pvr[:, mt2, :, 1, :], bvr[:, :, 1, :],
                                mybir.AluOpType.add)

                xk_t[0], xq_t[0], xv_t[0] = xk0, xq0, xv0
                dma_chunk(1)
                proj(0)

                ct_t = {}
                prev = None

                def bcast_norm(state):
                    # broadcast raw dens across partitions (PE), reciprocal
                    # in SBUF at partition base 0 (recip_approx_fast only
                    # works there), then normalize ctraw -> ct
                    qc, hp, ctraw, pp = state
                    pdup = ps.tile([128, QT_BLK], f32, name="pdup",
                                   tag="big", bufs=2)
                    nc.tensor.matmul(pdup[:], bsel_sb[:], dsb[0:65, pp, :],
                                     start=True, stop=True)
                    pbcs = crpool.tile([128, QT_BLK], f32, name="pbcs",
                                       tag="pbcs")
                    nc.vector.tensor_copy(pbcs[:], pdup[:])
                    pbcr = crpool.tile([128, QT_BLK], f32, name="pbcr",
                                       tag="pbcr")
                    nc.vector.reciprocal_approx_fast(pbcr[:], pbcs[:])
                    if hp == 0:
                        ct = ctpool.tile([128, 2, QT_BLK], bf16, name="ct",
                                         tag="ct")
                        ct_t[qc] = ct
                    ct = ct_t[qc]
                    nc.vector.tensor_tensor(ct[:, hp, :], ctraw[:, :],
                                            pbcr[:, :], mybir.AluOpType.mult)

                def wo(qc):
                    ct = ct_t[qc]
                    qsl = bass.ds(qc * QT_BLK, QT_BLK)
                    for jt in range(8):
                        pwo = ps.tile([128, QT_BLK], f32, name="pwo",
                                      tag="big", bufs=2)
                        for kc in range(2):
                            nc.tensor.matmul(
                                pwo[:], wo_sb[:, kc, bass.ds(jt * 128, 128)],
                                ct[:, kc, :], start=(kc == 0), stop=(kc == 1))
                        osb = opool.tile([128, QT_BLK], bf16, name="osb",
                                         tag="osb")
                        nc.scalar.activation(osb[:], pwo[:], IDF,
                                             bias=bo_sb[:, jt:jt + 1])
                        nc.sync.dma_start(out_d[jt][:, qsl], osb[:])

                for qc in range(N_QC):
                    for hp in range(2):
                        n_ki = 4 * qc + 4
                        pav_e = ps.tile([65, QT_BLK], f32, name="pav_e",
                                        tag="pave", bufs=1)
                        pav_o = ps.tile([128, QT_BLK], f32, name="pav_o",
                                        tag="pavo", bufs=1)

                        def av(idx, ki, et, lo):
                            alo = 0 if idx == 0 else lo
                            nc.tensor.matmul(
                                pav_e[:, alo:QT_BLK],
                                vaug_e[:, ki, hp, 0:65],
                                et[:, 0, alo:QT_BLK],
                                start=(idx == 0), stop=(idx == n_ki - 1),
                                skip_group_check=True)
                            nc.tensor.matmul(
                                pav_o[:, alo:QT_BLK],
                                vaug_o[:, ki, hp, :],
                                et[:, 1, alo:QT_BLK],
                                start=(idx == 0), stop=(idx == n_ki - 1),
                                skip_group_check=True)

                        # software pipeline 2 deep: AV(ki-2) runs while
                        # exp(ki-1) / affine(ki-1) and QK(ki) are in flight,
                        # hiding the slow gpsimd semaphore hop
                        pend = []
                        for ki in range(n_ki):
                            lo = max(0, 128 * ki - QT_BLK * qc)
                            st = ps.tile([128, 2, QT_BLK], f32, name="st",
                                         tag="st", bufs=2)
                            for side in range(2):
                                po = bass.ds(side * 64, 64)
                                nc.tensor.matmul(
                                    st[:, side, lo:QT_BLK],
                                    kt_sb[po, hp, bass.ds(ki * 128, 128)],
                                    qt_sb[po, hp,
                                          bass.ds(qc * QT_BLK + lo,
                                                  QT_BLK - lo)],
                                    start=True, stop=True)
                            et = epool.tile([128, 2, QT_BLK], bf16,
                                            name="et", tag="et")
                            nc.scalar.activation(et[:, :, lo:QT_BLK],
                                                 st[:, :, lo:QT_BLK], EXPF,
                                                 scale=0.125)
                            if ki >= 4 * qc:
                                # zero the causal staircase (cols lo..lo+127)
                                nc.gpsimd.affine_select(
                                    out=et[:, :, bass.ds(lo, 128)],
                                    in_=et[:, :, bass.ds(lo, 128)],
                                    compare_op=mybir.AluOpType.is_ge,
                                    fill=0.0, base=0,
                                    pattern=[[0, 2], [1, 128]],
                                    channel_multiplier=-1)
                            pend.append((ki, ki, et, lo))
                            if len(pend) > 2:
                                av(*pend.pop(0))
                        for p_ in pend:
                            av(*p_)

                        # denominators + raw C^T out (frees pav quickly)
                        ctraw = crpool.tile([128, QT_BLK], bf16,
                                            name="ctraw", tag="ctraw")
                        nc.vector.tensor_copy(ctraw[0:64, :], pav_e[0:64, :])
                        nc.vector.tensor_copy(ctraw[64:128, :],
                                              pav_o[64:128, :])
                        pp = hp
                        nc.vector.tensor_copy(dsb[64:65, pp, :],
                                              pav_e[64:65, :])
                        nc.vector.tensor_copy(dsb[0:1, pp, :],
                                              pav_o[0:1, :])
                        cur = (qc, hp, ctraw, pp)
                        if prev is not None:
                            bcast_norm(prev)
                        if hp == 0:
                            if qc < 3:
                                if qc + 2 < N_QC:
                                    dma_chunk(qc + 2)
                                proj(qc + 1)
                        else:
                            if qc > 0:
                                wo(qc - 1)
                        prev = cur

                bcast_norm(prev)
                wo(3)

    nc.compile()
    return nc


def kernel(query, key, value, mask, Wq, bq, Wk, bk, Wv, bv, Wo, bo):
    query = np.asarray(query, np.float32)
    key_ = np.asarray(key, np.float32)
    value = np.asarray(value, np.float32)
    Wq, Wk, Wv, Wo = (np.asarray(w, np.float32) for w in (Wq, Wk, Wv, Wo))
    bq, bk, bv, bo = (np.asarray(b_, np.float32) for b_ in (bq, bk, bv, bo))

    mask = np.asarray(mask)
    assert np.array_equal(mask != 0, np.tril(np.ones((S, S), bool))), \
        "kernel is specialized to the causal mask"
    if "nc" not in _CACHE:
        _CACHE["nc"] = _build()
    nc = _CACHE["nc"]

    def xt(x):  # [S, D] -> [N_QC, 128, KC, QT_BLK] bf16, partition-major
        a = x.T.reshape(KC, 128, S).transpose(1, 0, 2)  # [128, KC, S]
        a = a.reshape(128, KC, N_QC, QT_BLK).transpose(2, 0, 1, 3)
        return np.ascontiguousarray(a).astype(BF16)

    def wslice(W, c):  # [D, D] -> [128, KC, O] bf16 of W[o_slice].T
        hg = c % TP
        a = W[hg * O:(hg + 1) * O].T.reshape(KC, 128, O).transpose(1, 0, 2)
        return np.ascontiguousarray(a).astype(BF16)

    # bcast selector: rows 0-63 of pbc take 1/den_even (rdsb row 64),
    # rows 64-127 take 1/den_odd (rdsb row 0)
    bsel = np.zeros((65, 128), np.float32)
    bsel[64, 0:64] = 1.0
    bsel[0, 64:128] = 1.0

    in_maps = []
    for c in range(N_CORES):
        b_, hg = c // TP, c % TP
        osl = slice(hg * O, (hg + 1) * O)
        bo_part = bo if hg == 0 else np.zeros_like(bo)
        wot = Wo[:, osl].T.reshape(2, 128, D).transpose(1, 0, 2)
        in_maps.append({
            "xqt": xt(query[b_]),
            "xkt": xt(key_[b_]),
            "xvt": xt(value[b_]),
            "wqt": wslice(Wq, c),
            "wkt": wslice(Wk, c),
            "wvt": wslice(Wv, c),
            "wot": np.ascontiguousarray(wot).astype(BF16),
            "bqc": np.ascontiguousarray(bq[osl].reshape(2, 128).T),
            "bkc": np.ascontiguousarray(bk[osl].reshape(2, 128).T),
            "bvb": np.ascontiguousarray(np.broadcast_to(bv[osl], (128, O))),
            "boc": np.ascontiguousarray(bo_part.reshape(8, 128).T),
            "bsel": bsel,
        })

    res = run_bass_kernel_spmd(nc, in_maps, core_ids=list(range(N_CORES)))

    out = np.zeros((B, S, D), np.float32)
    for c in range(N_CORES):
        part = res.results[c]["out"].reshape(D, S)  # out^T [j, s]
        out[c // TP] += part.T.astype(np.float32)
    return out
